# revision 1
# baseline (speedup 1.0000x reference)
"""APPNP GNN (MLP -> K iterations of normalized sparse aggregation -> log_softmax)
on 8 Trainium2 NeuronCores via Bass/Tile.

Distribution: 1D destination-node sharding. Each core owns N/8 destination
rows. Edges are partitioned by destination core, dst-sorted into windows of
128 destination rows, and sub-grouped by source range (4 ranges of N/4 rows so
gather indices fit int16). Per APPNP iteration each core:
  - dma_gather's the source rows of its edges (256B rows) from a replicated
    y = D^-1/2 x buffer,
  - segment-sums them into PSUM windows with one-hot selection-matrix matmuls,
  - applies x' = (1-a) * dinv * (psum + y_own) + a*h0,
  - AllGathers the new y slice so every core has the full y for the next
    iteration.
The MLP front (x @ W0 -> relu -> @ W1) and the final log_softmax run on
device as well. All graph structure (counts/offsets) is baked into the NEFF
at build time; index/selection data are runtime inputs.
"""

import math
import numpy as np

import concourse.bass as bass
import concourse.bacc as bacc
import concourse.mybir as mybir
import concourse.tile as tile
from concourse.bass_utils import run_bass_kernel_spmd
from concourse._compat import cdiv

F32 = mybir.dt.float32
FP8 = mybir.dt.float8e4
BF16 = mybir.dt.bfloat16
I16 = mybir.dt.int16
AF = mybir.ActivationFunctionType
ALU = mybir.AluOpType

P = 128


class Cfg:
    def __init__(self, N, E, F_IN, HID, C, K, ALPHA, ncores=8, nranges=4,
                 batch_windows=7, sgroup=8, debug=False, max_call_idx=4096,
                 dma_scratch=49152):
        self.debug = debug
        self.max_call_idx = max_call_idx
        self.dma_scratch = dma_scratch
        self.N, self.E, self.F_IN, self.HID, self.C = N, E, F_IN, HID, C
        self.K, self.ALPHA = K, ALPHA
        self.ncores = ncores
        self.rows = N // ncores                 # rows per core
        assert self.rows * ncores == N
        self.nwin = cdiv(self.rows, P)          # dst windows per core
        self.rows_pad = self.nwin * P
        self.nranges = nranges
        self.rng_rows = cdiv(N, nranges)        # source rows per range
        assert self.rng_rows <= 32768
        self.BW = batch_windows                 # windows per batch
        self.nbatch = cdiv(self.nwin, batch_windows)
        self.sgroup = sgroup                    # chunks per S-build group


# ---------------------------------------------------------------------------
# Host preprocessing: graph structure -> uniform compile-time layout + per-core
# runtime index data.
# ---------------------------------------------------------------------------

def preprocess(cfg, edge_index):
    src = np.asarray(edge_index[0], dtype=np.int64)
    dst = np.asarray(edge_index[1], dtype=np.int64)
    N, ncores = cfg.N, cfg.ncores

    deg = np.bincount(dst, minlength=N).astype(np.float64) + 1.0
    dinv = (1.0 / np.sqrt(deg)).astype(np.float32)

    core = dst // cfg.rows
    w = (dst % cfg.rows) // P
    r = src // cfg.rng_rows
    gid = (core * cfg.nwin + w) * cfg.nranges + r
    order = np.argsort(gid, kind="stable")
    gid_s = gid[order]
    src_s = src[order]
    dst_s = dst[order]

    ngroup = ncores * cfg.nwin * cfg.nranges
    gcounts = np.bincount(gid_s, minlength=ngroup).reshape(
        ncores, cfg.nwin, cfg.nranges)
    # uniform structure: chunks per (window, range) = max over cores
    nchunk_wr = np.ceil(gcounts / P).astype(np.int64).max(axis=0)  # [nwin, nranges]

    # layout of the padded per-core edge stream:
    # for b in batches: for r in ranges: for w in windows(b): group slots.
    # Each (b, r) stream is split into gather sub-calls of <= max_call_idx
    # indices (chunk-aligned).
    slot_ofs = np.zeros((cfg.nwin, cfg.nranges), dtype=np.int64)
    calls = []   # sub-calls: dict(b, r, pos, n, chunk0, chunks=[(ck, w)...])
    pos = 0
    chunkpos = 0
    cap_ck = cfg.max_call_idx // P
    for b in range(cfg.nbatch):
        wlist = list(range(b * cfg.BW, min((b + 1) * cfg.BW, cfg.nwin)))
        for r in range(cfg.nranges):
            # chunk->window sequence for this (b, r)
            seq = []
            for w in wlist:
                nck = int(nchunk_wr[w, r])
                slot_ofs[w, r] = pos + len(seq) * P
                seq.extend([w] * nck)
            for s0 in range(0, len(seq), cap_ck):
                grp = seq[s0:s0 + cap_ck]
                calls.append(dict(
                    b=b, r=r, pos=pos + s0 * P, n=len(grp) * P,
                    q=len(calls) % 4,
                    chunk0=chunkpos + s0,
                    chunks=[(chunkpos + s0 + i, w) for i, w in enumerate(grp)]))
            pos += len(seq) * P
            chunkpos += len(seq)
    L = pos                       # padded stream length (same for all cores)
    NCHUNKS = chunkpos

    # per-batch first/last chunk flags: one PSUM accumulation group per batch
    first_chunk = {}
    last_chunk = {}
    for b in range(cfg.nbatch):
        cks = [ck for c in calls if c["b"] == b for ck, _ in c["chunks"]]
        assert cks, f"batch {b} has no chunks"
        first_chunk[b] = min(cks)
        last_chunk[b] = max(cks)

    # scatter each edge into its padded position
    flat_counts = gcounts.reshape(-1)
    gstart = np.zeros(ngroup + 1, dtype=np.int64)
    np.cumsum(flat_counts, out=gstart[1:])
    rank = np.arange(len(src_s), dtype=np.int64) - gstart[gid_s]
    core_s = gid_s // (cfg.nwin * cfg.nranges)
    wr_s = gid_s % (cfg.nwin * cfg.nranges)
    pos_s = slot_ofs.reshape(-1)[wr_s] + rank

    idx_pad = np.zeros((ncores, L), dtype=np.int16)
    slot_pad = np.full((ncores, L), 200.0, dtype=np.float32)
    idx_pad[core_s, pos_s] = (src_s - (src_s // cfg.rng_rows) * cfg.rng_rows
                              ).astype(np.int16)
    slot_pad[core_s, pos_s] = (dst_s % cfg.rows % P).astype(np.float32)

    # dstslot tensor [128, NCHUNKS]
    dstslot = np.transpose(slot_pad.reshape(ncores, NCHUNKS, P), (0, 2, 1)).copy()

    # idx tensor: per batch a [128, maxcols_b] block; range r occupies
    # partitions 32r..32r+31 (16-row wrap, replicated twice). Blocks are
    # concatenated along columns.
    batch_cols = []
    for b in range(cfg.nbatch):
        cur = [0, 0, 0, 0]
        for c in calls:
            if c["b"] == b and c["n"] > 0:
                c["bandcol"] = cur[c["q"]]
                cur[c["q"]] += c["n"] // 16
        batch_cols.append(max(cur))
    TOTCOLS = int(np.sum(batch_cols))
    idx_t = np.zeros((ncores, 128, TOTCOLS), dtype=np.int16)
    bc_ofs = np.concatenate([[0], np.cumsum(batch_cols)]).astype(np.int64)
    for call in calls:
        b, q, p0, n = call["b"], call["q"], call["pos"], call["n"]
        if n == 0:
            continue
        seg = idx_pad[:, p0:p0 + n]                     # [ncores, n]
        wrap = seg.reshape(ncores, n // 16, 16).transpose(0, 2, 1)  # [nc,16,cols]
        c0 = int(bc_ofs[b] + call["bandcol"])
        idx_t[:, 32 * q:32 * q + 16, c0:c0 + n // 16] = wrap
        idx_t[:, 32 * q + 16:32 * q + 32, c0:c0 + n // 16] = wrap
        call["col0"] = int(c0)

    # per-batch half-split points for S streaming
    halfmid = {}
    for b in range(cfg.nbatch):
        f, l = first_chunk[b], last_chunk[b]
        halfmid[b] = f + (l - f + 2) // 2
    meta = dict(calls=calls, L=L, NCHUNKS=NCHUNKS, TOTCOLS=TOTCOLS,
                batch_cols=batch_cols, bc_ofs=bc_ofs,
                first_chunk=first_chunk, last_chunk=last_chunk,
                halfmid=halfmid, nchunk_wr=nchunk_wr)
    import ml_dtypes
    s8 = np.empty((ncores, 128, NCHUNKS * 128), dtype=ml_dtypes.float8_e4m3)
    dstcols = np.arange(128, dtype=np.float32)[None, None, :]
    for c in range(ncores):
        oh = (dstslot[c][:, :, None] == dstcols)
        s8[c] = oh.astype(ml_dtypes.float8_e4m3).reshape(128, NCHUNKS * 128)
    return dinv, idx_t, dstslot, s8, meta


# ---------------------------------------------------------------------------
# Kernel build
# ---------------------------------------------------------------------------

def build(cfg, meta):
    nc = bacc.Bacc("TRN2", target_bir_lowering=False,
                   num_swdge_queues=4,
                   dynamic_dma_scratch_size=cfg.dma_scratch)
    NCHUNKS, TOTCOLS = meta["NCHUNKS"], meta["TOTCOLS"]
    calls, bc_ofs = meta["calls"], meta["bc_ofs"]
    first_chunk, last_chunk = meta["first_chunk"], meta["last_chunk"]
    halfmid = meta["halfmid"]
    C, HID, F_IN = cfg.C, cfg.HID, cfg.F_IN
    nwin, BW, nbatch = cfg.nwin, cfg.BW, cfg.nbatch
    KF = F_IN // P                    # k-tiles in layer 1

    xin = nc.dram_tensor("xin", [cfg.rows_pad, F_IN], F32, kind="ExternalInput")
    w0 = nc.dram_tensor("w0", [F_IN, HID], F32, kind="ExternalInput")
    w1 = nc.dram_tensor("w1", [HID, C], F32, kind="ExternalInput")
    b0c = nc.dram_tensor("b0c", [P, 1], F32, kind="ExternalInput")
    b1r = nc.dram_tensor("b1r", [P, C], F32, kind="ExternalInput")
    dinv_in = nc.dram_tensor("dinv_in", [P, nwin], F32, kind="ExternalInput")
    dinvs_in = nc.dram_tensor("dinvs_in", [P, nwin], F32, kind="ExternalInput")
    ident_in = nc.dram_tensor("ident_in", [P, P], F32, kind="ExternalInput")
    idxs_in = nc.dram_tensor("idxs_in", [128, TOTCOLS], I16, kind="ExternalInput")
    s8_in = nc.dram_tensor("s8_in", [128, NCHUNKS * 128], FP8,
                           kind="ExternalInput")
    out = nc.dram_tensor("out", [cfg.rows_pad, C], F32, kind="ExternalOutput")
    if cfg.debug:
        dbg_y0 = nc.dram_tensor("dbg_y0", [cfg.rows_pad, C], F32,
                                kind="ExternalOutput")
        dbg_x1 = nc.dram_tensor("dbg_x1", [cfg.rows_pad, C], F32,
                                kind="ExternalOutput")

    rg = [list(range(cfg.ncores))]

    with tile.TileContext(nc) as tc:
        with tc.tile_pool(name="const", bufs=1) as cp, \
             tc.tile_pool(name="resid", bufs=1) as rp, \
             tc.tile_pool(name="dram", bufs=2, space="DRAM") as dp:

            # ---- constants / residents ----
            identSB = cp.tile([P, P], F32)
            nc.sync.dma_start(identSB[:], ident_in[:])
            w0SB = cp.tile([P, KF, HID], F32)
            nc.sync.dma_start(w0SB[:], w0[:].rearrange("(k p) h -> p k h", p=P))
            w1SB = cp.tile([P, C], F32)
            nc.sync.dma_start(w1SB[:], w1[:])
            b0SB = cp.tile([P, 1], F32)
            nc.sync.dma_start(b0SB[:], b0c[:])
            b1SB = cp.tile([P, C], F32)
            nc.sync.dma_start(b1SB[:], b1r[:])
            dinvSB = cp.tile([P, nwin], F32)
            nc.sync.dma_start(dinvSB[:], dinv_in[:])
            dinvsSB = cp.tile([P, nwin], F32)
            nc.sync.dma_start(dinvsSB[:], dinvs_in[:])

            ySB = rp.tile([P, nwin * C], F32)      # own slice of y, window-major
            h0aSB = rp.tile([P, nwin * C], F32)    # alpha * h0

            # =========== MLP phase ===========
            ag0 = dp.tile([cfg.rows, C], F32)
            with tc.tile_pool(name="mlpw", bufs=3) as wp, \
                 tc.tile_pool(name="mlpp", bufs=2, space="PSUM") as pp:
                for t in range(nwin):
                    xt = wp.tile([P, F_IN], F32, tag="xt")
                    nc.sync.dma_start(xt[:], xin[t * P:(t + 1) * P, :])
                    hT_ps = pp.tile([P, P], F32, tag="hT")
                    for k in range(KF):
                        xT_ps = pp.tile([P, P], F32, tag="xT")
                        nc.tensor.transpose(out=xT_ps[:],
                                            in_=xt[:, k * P:(k + 1) * P],
                                            identity=identSB[:])
                        xT = wp.tile([P, P], F32, tag="xTs")
                        nc.vector.tensor_copy(xT[:], xT_ps[:])
                        nc.tensor.matmul(out=hT_ps[:], lhsT=w0SB[:, k, :],
                                         rhs=xT[:], start=(k == 0),
                                         stop=(k == KF - 1))
                    hT = wp.tile([P, P], F32, tag="hTs")
                    nc.scalar.activation(hT[:], hT_ps[:], AF.Relu,
                                         bias=b0SB[:, 0:1], scale=1.0)
                    h2_ps = pp.tile([P, C], F32, tag="h2")
                    nc.tensor.matmul(out=h2_ps[:], lhsT=hT[:], rhs=w1SB[:],
                                     start=True, stop=True)
                    ysl = ySB[:, t * C:(t + 1) * C]
                    h0sl = h0aSB[:, t * C:(t + 1) * C]
                    h0t = wp.tile([P, C], F32, tag="h0t")
                    nc.vector.tensor_tensor(h0t[:], h2_ps[:], b1SB[:], op=ALU.add)
                    nc.vector.tensor_scalar_mul(h0sl, h0t[:], cfg.ALPHA)
                    nc.vector.tensor_scalar(ysl, h0t[:], dinvSB[:, t:t + 1],
                                            None, ALU.mult)
                # write y slice -> ag0
                _dma_slice_to_dram(nc, ag0, ySB, cfg, 0, nwin)
                if cfg.debug:
                    for t in range(nwin):
                        nc.sync.dma_start(dbg_y0[t * P:(t + 1) * P, :],
                                          ySB[:, t * C:(t + 1) * C])

            yfull = dp.tile([cfg.N, C], F32, addr_space="Shared", tag="yfull")
            nc.gpsimd.collective_compute(
                "AllGather", ALU.bypass, replica_groups=rg,
                ins=[ag0[:].opt()], outs=[yfull[:].opt()])

            # =========== APPNP iterations ===========
            for it in range(cfg.K):
                last_it = (it == cfg.K - 1)
                if not last_it:
                    agin = dp.tile([cfg.rows, C], F32, tag="agin")
                with tc.tile_pool(name="gpool", bufs=3) as gp, \
                     tc.tile_pool(name="ipool", bufs=3) as ip, \
                     tc.tile_pool(name="spool", bufs=2) as sp, \
                     tc.tile_pool(name="wk", bufs=6) as wk, \
                     tc.tile_pool(name="pp", bufs=2, space="PSUM") as pp:
                    for b in range(nbatch):
                        wlist = list(range(b * BW, min((b + 1) * BW, nwin)))
                        bcalls = [c for c in calls if c["b"] == b]
                        # load this batch's idx block
                        cols_b = meta["batch_cols"][b]
                        idxT = ip.tile([128, cols_b], I16, tag="idx")
                        nc.sync.dma_start(
                            idxT[:], idxs_in[:, bc_ofs[b]:bc_ofs[b] + cols_b])
                        # stream this batch's S one-hots (fp8) in two halves
                        f0, l0, mid = first_chunk[b], last_chunk[b], halfmid[b]
                        sA = sp.tile([P, (mid - f0) * P], FP8, tag="sA")
                        nc.sync.dma_start(sA[:], s8_in[:, f0 * P:mid * P])
                        sB = sp.tile([P, (l0 + 1 - mid) * P], FP8, tag="sB")
                        nc.sync.dma_start(sB[:], s8_in[:, mid * P:(l0 + 1) * P])
                        psumB = pp.tile([P, len(wlist) * C], F32, tag="ps")
                        for call in bcalls:
                            n = call["n"]
                            if n == 0:
                                continue
                            q = call["q"]
                            r = call["r"]
                            col0 = call["col0"] - bc_ofs[b]
                            gt = gp.tile([P, (n // P) * C], F32, tag="G")
                            src_view = yfull[r * cfg.rng_rows:
                                             min((r + 1) * cfg.rng_rows, cfg.N), :]
                            nc.gpsimd.dma_gather(
                                gt[:].rearrange("p (c f) -> p c f", f=C),
                                src_view, idxT[:, col0:col0 + n // 16], n, n, C,
                                queue_num=q, single_packet=False)
                            gb = gp.tile([P, (n // P) * C], BF16, tag="GB")
                            nc.vector.tensor_copy(gb[:], gt[:])
                            # matmuls for this sub-call's chunks
                            for j, (ck, w) in enumerate(call["chunks"]):
                                wl = w - b * BW
                                if ck < mid:
                                    ssl = sA[:, (ck - f0) * P:(ck - f0 + 1) * P]
                                else:
                                    ssl = sB[:, (ck - mid) * P:(ck - mid + 1) * P]
                                nc.tensor.matmul(
                                    out=psumB[:, wl * C:(wl + 1) * C],
                                    lhsT=ssl,
                                    rhs=gb[:, j * C:(j + 1) * C],
                                    start=(ck == first_chunk[b]),
                                    stop=(ck == last_chunk[b]))
                        # epilogue per window
                        for w in wlist:
                            wl = w - b * BW
                            ysl = ySB[:, w * C:(w + 1) * C]
                            h0sl = h0aSB[:, w * C:(w + 1) * C]
                            t1 = wk.tile([P, C], F32, tag="t1")
                            nc.vector.tensor_tensor(
                                t1[:], psumB[:, wl * C:(wl + 1) * C], ysl,
                                op=ALU.add)
                            t2 = wk.tile([P, C], F32, tag="t2")
                            nc.scalar.activation(t2[:], t1[:], AF.Copy,
                                                 bias=0.0,
                                                 scale=dinvsSB[:, w:w + 1])
                            xn = wk.tile([P, C], F32, tag="xn")
                            nc.vector.tensor_tensor(xn[:], t2[:], h0sl,
                                                    op=ALU.add)
                            if cfg.debug and it == 0:
                                nc.sync.dma_start(
                                    dbg_x1[w * P:(w + 1) * P, :], xn[:])
                            if not last_it:
                                nc.scalar.activation(ysl, xn[:], AF.Copy,
                                                     bias=0.0,
                                                     scale=dinvSB[:, w:w + 1])
                            else:
                                _log_softmax(nc, wk, xn, out, w, cfg)
                        if not last_it:
                            _dma_batch_to_dram(nc, agin, ySB, cfg, b, wlist)
                if not last_it:
                    yfull = dp.tile([cfg.N, C], F32, addr_space="Shared",
                                    tag="yfull")
                    nc.gpsimd.collective_compute(
                        "AllGather", ALU.bypass, replica_groups=rg,
                        ins=[agin[:].opt()], outs=[yfull[:].opt()])

    nc.compile()
    return nc


def _dma_slice_to_dram(nc, dram_t, ySB, cfg, w0_, nwins):
    """Copy windows [w0_, w0_+nwins) of window-major ySB into row-major dram
    tensor rows [w0_*P ...], clipping at cfg.rows."""
    C = cfg.C
    wfull = nwins
    # clip to full windows + remainder
    end_row = min((w0_ + nwins) * P, cfg.rows)
    n_full = (end_row - w0_ * P) // P
    if n_full > 0:
        dv = dram_t[w0_ * P: w0_ * P + n_full * P, :].rearrange(
            "(w p) c -> p w c", p=P)
        sv = ySB[:, w0_ * C:(w0_ + n_full) * C].rearrange(
            "p (w c) -> p w c", c=C)
        nc.sync.dma_start(dv, sv)
    rem = end_row - (w0_ * P + n_full * P)
    if rem > 0:
        w = w0_ + n_full
        nc.sync.dma_start(dram_t[w * P:w * P + rem, :],
                          ySB[0:rem, w * C:(w + 1) * C])


def _dma_batch_to_dram(nc, dram_t, ySB, cfg, b, wlist):
    _dma_slice_to_dram(nc, dram_t, ySB, cfg, wlist[0], len(wlist))


def _log_softmax(nc, wk, xn, out, w, cfg):
    C = cfg.C
    negm = wk.tile([P, 1], F32, tag="negm")
    nc.vector.reduce_max(negm[:], xn[:], axis=mybir.AxisListType.X,
                         negate=True)
    e = wk.tile([P, C], F32, tag="e")
    ssum = wk.tile([P, 1], F32, tag="ssum")
    nc.scalar.activation(e[:], xn[:], AF.Exp, bias=negm[:, 0:1], scale=1.0,
                         accum_out=ssum[:])
    lse = wk.tile([P, 1], F32, tag="lse")
    nc.scalar.activation(lse[:], ssum[:], AF.Ln)
    res = wk.tile([P, C], F32, tag="res")
    nc.vector.tensor_scalar(res[:], xn[:], negm[:, 0:1], lse[:, 0:1],
                            ALU.add, ALU.subtract)
    nc.sync.dma_start(out[w * P:(w + 1) * P, :], res[:])


# ---------------------------------------------------------------------------
# Host-side driver
# ---------------------------------------------------------------------------

_BUILD_CACHE = {}


def _get_kernel(cfg, edge_index):
    key = hash(edge_index.tobytes()) ^ hash((cfg.N, cfg.E, cfg.K))
    if key in _BUILD_CACHE:
        return _BUILD_CACHE[key]
    dinv, idx_t, dstslot, s8, meta = preprocess(cfg, edge_index)
    nc = build(cfg, meta)
    _BUILD_CACHE[key] = (nc, dinv, idx_t, s8, meta)
    return _BUILD_CACHE[key]


def run(cfg, inputs, edge_index, W0, b0, W1, b1, trace=False):
    nc, dinv, idx_t, s8, meta = _get_kernel(cfg, edge_index)

    ident = np.eye(P, dtype=np.float32)
    b0c = np.asarray(b0, np.float32).reshape(P, 1)
    b1r = np.tile(np.asarray(b1, np.float32)[None, :], (P, 1))
    W0 = np.asarray(W0, np.float32)
    W1 = np.asarray(W1, np.float32)
    x = np.asarray(inputs, np.float32)

    in_maps = []
    for c in range(cfg.ncores):
        r0 = c * cfg.rows
        xs = np.zeros((cfg.rows_pad, cfg.F_IN), np.float32)
        xs[:cfg.rows] = x[r0:r0 + cfg.rows]
        tmp = np.zeros(cfg.rows_pad, np.float32)
        tmp[:cfg.rows] = dinv[r0:r0 + cfg.rows]
        dv = tmp.reshape(cfg.nwin, P).T.copy()   # dv[p, w] = dinv[r0 + w*P + p]
        dvs = dv * (1.0 - cfg.ALPHA)
        in_maps.append(dict(
            xin=xs, w0=W0, w1=W1, b0c=b0c, b1r=b1r,
            dinv_in=dv, dinvs_in=dvs, ident_in=ident,
            idxs_in=idx_t[c], s8_in=s8[c]))

    res = run_bass_kernel_spmd(nc, in_maps, core_ids=list(range(cfg.ncores)),
                               trace=trace)
    outs = [res.results[c]["out"][:cfg.rows] for c in range(cfg.ncores)]
    return np.concatenate(outs, axis=0), res


def kernel(inputs, edge_index, W0, b0, W1, b1):
    cfg = Cfg(N=100000, E=3200000, F_IN=256, HID=128, C=64, K=10, ALPHA=0.1)
    out, _ = run(cfg, np.asarray(inputs), np.asarray(edge_index),
                 W0, b0, W1, b1)
    return out



# revision 11
# speedup vs baseline: 2.0448x; 2.0448x over previous
"""APPNP GNN (MLP -> K iterations of normalized sparse aggregation -> log_softmax)
on 8 Trainium2 NeuronCores via Bass/Tile. V2.

Distribution: 1D destination-node sharding; each core owns N/8 destination
rows. Per APPNP iteration each core dma_gathers the source rows of its edges
(256B bf16-padded rows) from a replicated y = D^-1/2 x buffer and
segment-sums them into PSUM with one-hot fp8 selection-matrix matmuls.

V2 layout/optimizations over the original baseline:
  - y is stored bf16 in 256B padded rows; the gather output feeds the PE
    matmuls directly (no DVE cast -> no DVE/GpSimd SBUF port-lock contention
    with SWDGE descriptor generation).
  - Edge streams are packed per (batch, range) with chunks spanning window
    boundaries (pad ~2.5% instead of 12.5% -> fewer gather descriptors).
  - The self-loop term and the alpha*h0 teleport term are folded into the
    PSUM accumulation as fp8-identity matmuls (rhs = resident bf16 y and
    z = alpha*h0/((1-a)*dinv)), so the per-window epilogue is a single
    scalar-engine activation; the iteration loop issues no DVE work at all.
All graph structure is baked into the NEFF at build time; index/one-hot
data are runtime inputs.
"""

import numpy as np

import concourse.bass as bass
import concourse.bacc as bacc
import concourse.mybir as mybir
import concourse.tile as tile
from concourse.bass_utils import run_bass_kernel_spmd
from concourse._compat import cdiv

F32 = mybir.dt.float32
FP8 = mybir.dt.float8e4
BF16 = mybir.dt.bfloat16
I16 = mybir.dt.int16
AF = mybir.ActivationFunctionType
ALU = mybir.AluOpType

P = 128


class Cfg:
    def __init__(self, N, E, F_IN, HID, C, K, ALPHA, ncores=8, nranges=4,
                 batch_windows=7, max_call_idx=4096, dma_scratch=49152):
        self.max_call_idx = max_call_idx
        self.dma_scratch = dma_scratch
        self.N, self.E, self.F_IN, self.HID, self.C = N, E, F_IN, HID, C
        self.K, self.ALPHA = K, ALPHA
        self.ncores = ncores
        self.rows = N // ncores                 # rows per core
        assert self.rows * ncores == N
        self.nwin = cdiv(self.rows, P)          # dst windows per core
        self.rows_pad = self.nwin * P
        self.nranges = nranges
        self.rng_rows = cdiv(N, nranges)        # source rows per range
        assert self.rng_rows <= 32768
        self.BW = batch_windows                 # windows per batch
        self.nbatch = cdiv(self.nwin, batch_windows)


# ---------------------------------------------------------------------------
# Host preprocessing: graph structure -> uniform compile-time layout + per-core
# runtime index/one-hot data.
# ---------------------------------------------------------------------------

def preprocess(cfg, edge_index):
    src = np.asarray(edge_index[0], dtype=np.int64)
    dst = np.asarray(edge_index[1], dtype=np.int64)
    N, ncores = cfg.N, cfg.ncores
    nbatch, nranges, BW = cfg.nbatch, cfg.nranges, cfg.BW

    deg = np.bincount(dst, minlength=N).astype(np.float64) + 1.0
    dinv = (1.0 / np.sqrt(deg)).astype(np.float32)

    core = dst // cfg.rows
    w = (dst % cfg.rows) // P                     # window within core
    b = w // BW                                   # batch within core
    r = src // cfg.rng_rows                       # source range
    gid = (core * nbatch + b) * nranges + r
    # stable sort by (core, b, r) then window
    order = np.argsort(gid * cfg.nwin + w, kind="stable")
    gid_s = gid[order]
    src_s = src[order]
    dst_s = dst[order]
    w_s = w[order]

    ngroup = ncores * nbatch * nranges
    gcounts = np.bincount(gid_s, minlength=ngroup).reshape(
        ncores, nbatch, nranges)
    # uniform padded length per (b, r) = max over cores, chunk-aligned
    Lbr = (np.ceil(gcounts.max(axis=0) / P) * P).astype(np.int64)  # [nbatch,nranges]

    # stream layout: for b in batches: for r in ranges: Lbr[b, r] positions
    grp_ofs = np.zeros((nbatch, nranges), dtype=np.int64)
    pos = 0
    for bb in range(nbatch):
        for rr in range(nranges):
            grp_ofs[bb, rr] = pos
            pos += Lbr[bb, rr]
    L = pos
    NCHUNKS = L // P

    # scatter each edge into its padded stream position
    flat_counts = gcounts.reshape(ncores, -1)
    gstart = np.zeros((ncores, nbatch * nranges + 1), dtype=np.int64)
    np.cumsum(flat_counts, axis=1, out=gstart[:, 1:])
    core_base = np.zeros(ncores + 1, dtype=np.int64)
    np.cumsum(flat_counts.sum(axis=1), out=core_base[1:])
    br_s = gid_s % (nbatch * nranges)
    core_s = gid_s // (nbatch * nranges)
    rank = (np.arange(len(src_s), dtype=np.int64) - core_base[core_s]
            - gstart[core_s, br_s])
    pos_s = grp_ofs.reshape(-1)[br_s] + rank

    idx_pad = np.zeros((ncores, L), dtype=np.int16)
    win_pad = np.full((ncores, L), -1, dtype=np.int64)     # window, -1 = pad
    slot_pad = np.zeros((ncores, L), dtype=np.int64)       # dst slot in window
    idx_pad[core_s, pos_s] = (src_s - (src_s // cfg.rng_rows) * cfg.rng_rows
                              ).astype(np.int16)
    win_pad[core_s, pos_s] = w_s
    slot_pad[core_s, pos_s] = (dst_s % cfg.rows) % P

    # ---- piece enumeration (uniform across cores) ----
    # For chunk ck (rows [128ck,128ck+128) of the stream), the set of windows
    # present must be computed across ALL cores so the piece structure is
    # uniform; a piece is (chunk, window). Emission order: batch-major,
    # range-major, chunk asc, window asc.
    pieces = []          # list of dict(ck, w, b, r)
    chunk_pieces = [[] for _ in range(NCHUNKS)]
    calls = []           # dict(b, r, pos, n, q, chunks=[ck...], col0, bandcol)
    cap_ck = cfg.max_call_idx // P
    piece_cnt_per_win = np.zeros((ncores if False else 1,), dtype=np.int64)
    for bb in range(nbatch):
        for rr in range(nranges):
            p0 = grp_ofs[bb, rr]
            nck = Lbr[bb, rr] // P
            ck0 = p0 // P
            # windows present per chunk across all cores
            for c_i in range(nck):
                ck = ck0 + c_i
                seg_w = win_pad[:, ck * P:(ck + 1) * P]
                wins = np.unique(seg_w)
                wins = wins[wins >= 0]
                if len(wins) == 0:
                    # fully padded chunk (possible on low-count cores at the
                    # group tail on SOME cores but uniform structure needs at
                    # least one piece; attach to the batch's first window)
                    wins = np.array([bb * BW], dtype=np.int64)
                for wv in wins:
                    pc = dict(ck=int(ck), w=int(wv), b=bb, r=rr,
                              sb=len(pieces))
                    pieces.append(pc)
                    chunk_pieces[ck].append(pc)
            # gather sub-calls (chunk-aligned, <= max_call_idx idxs)
            for s0 in range(0, nck, cap_ck):
                grp = list(range(ck0 + s0, ck0 + min(s0 + cap_ck, nck)))
                calls.append(dict(
                    b=bb, r=rr, pos=int((ck0 + s0) * P),
                    n=len(grp) * P, q=len(calls) % 4, chunks=grp))
    NPIECES = len(pieces)

    # first/last piece per (core-uniform) window for PSUM start/stop flags
    first_piece = {}
    last_piece = {}
    for pc in pieces:
        wv = pc["w"]
        if wv not in first_piece:
            first_piece[wv] = pc["sb"]
        last_piece[wv] = pc["sb"]

    # per-batch piece ranges for S streaming
    batch_sb = {}
    for pc in pieces:
        bb = pc["b"]
        lo, hi = batch_sb.get(bb, (pc["sb"], pc["sb"]))
        batch_sb[bb] = (min(lo, pc["sb"]), max(hi, pc["sb"]))

    # ---- S one-hot blocks (fp8) ----
    import ml_dtypes
    s8 = np.zeros((ncores, 128, NPIECES * 128), dtype=ml_dtypes.float8_e4m3)
    lanes = np.arange(P)
    for pc in pieces:
        ck, wv, sb = pc["ck"], pc["w"], pc["sb"]
        seg_w = win_pad[:, ck * P:(ck + 1) * P]      # [ncores, 128]
        seg_s = slot_pad[:, ck * P:(ck + 1) * P]     # [ncores, 128]
        mask = seg_w == wv                           # [ncores, 128]
        # one-hot: s8[c, lane, sb*128 + slot] = 1 where mask
        cc, ll = np.nonzero(mask)
        s8[cc, ll, sb * 128 + seg_s[cc, ll]] = 1.0

    # ---- idx tensor: per batch a [128, cols_b] block; call idx at band
    # 32q..32q+32 (16-row wrap, replicated twice), concatenated per batch ----
    batch_cols = []
    for bb in range(nbatch):
        cur = [0, 0, 0, 0]
        for c in calls:
            if c["b"] == bb and c["n"] > 0:
                c["bandcol"] = cur[c["q"]]
                cur[c["q"]] += c["n"] // 16
        batch_cols.append(max(cur))
    TOTCOLS = int(np.sum(batch_cols))
    idx_t = np.zeros((ncores, 128, TOTCOLS), dtype=np.int16)
    bc_ofs = np.concatenate([[0], np.cumsum(batch_cols)]).astype(np.int64)
    for call in calls:
        bb, q, p0, n = call["b"], call["q"], call["pos"], call["n"]
        if n == 0:
            continue
        seg = idx_pad[:, p0:p0 + n]                     # [ncores, n]
        wrap = seg.reshape(ncores, n // 16, 16).transpose(0, 2, 1)
        c0 = int(bc_ofs[bb] + call["bandcol"])
        idx_t[:, 32 * q:32 * q + 16, c0:c0 + n // 16] = wrap
        idx_t[:, 32 * q + 16:32 * q + 32, c0:c0 + n // 16] = wrap
        call["col0"] = int(c0)

    # per-batch half-split (by piece index) for S streaming
    halfmid = {}
    for bb in range(nbatch):
        f, l = batch_sb[bb]
        halfmid[bb] = f + (l - f + 2) // 2
    meta = dict(calls=calls, L=L, NCHUNKS=NCHUNKS, NPIECES=NPIECES,
                TOTCOLS=TOTCOLS, batch_cols=batch_cols, bc_ofs=bc_ofs,
                chunk_pieces=chunk_pieces, first_piece=first_piece,
                last_piece=last_piece, batch_sb=batch_sb, halfmid=halfmid)
    return dinv, idx_t, s8, meta


# ---------------------------------------------------------------------------
# Kernel build
# ---------------------------------------------------------------------------

def build(cfg, meta):
    nc = bacc.Bacc("TRN2", target_bir_lowering=False,
                   num_swdge_queues=4,
                   dynamic_dma_scratch_size=cfg.dma_scratch)
    NPIECES, TOTCOLS = meta["NPIECES"], meta["TOTCOLS"]
    calls, bc_ofs = meta["calls"], meta["bc_ofs"]
    chunk_pieces = meta["chunk_pieces"]
    first_piece, last_piece = meta["first_piece"], meta["last_piece"]
    batch_sb, halfmid = meta["batch_sb"], meta["halfmid"]
    C, HID, F_IN = cfg.C, cfg.HID, cfg.F_IN
    C2 = 2 * C
    nwin, BW, nbatch = cfg.nwin, cfg.BW, cfg.nbatch
    KF = F_IN // P                    # k-tiles in layer 1

    xin = nc.dram_tensor("xin", [cfg.rows_pad, F_IN], F32, kind="ExternalInput")
    w0 = nc.dram_tensor("w0", [F_IN, HID], F32, kind="ExternalInput")
    w1 = nc.dram_tensor("w1", [HID, C], F32, kind="ExternalInput")
    b0c = nc.dram_tensor("b0c", [P, 1], F32, kind="ExternalInput")
    b1r = nc.dram_tensor("b1r", [P, C], F32, kind="ExternalInput")
    dinv_in = nc.dram_tensor("dinv_in", [P, nwin], F32, kind="ExternalInput")
    dinvs_in = nc.dram_tensor("dinvs_in", [P, nwin], F32, kind="ExternalInput")
    dd_in = nc.dram_tensor("dd_in", [P, nwin], F32, kind="ExternalInput")
    zs_in = nc.dram_tensor("zs_in", [P, nwin], F32, kind="ExternalInput")
    ident_in = nc.dram_tensor("ident_in", [P, P], F32, kind="ExternalInput")
    ident8_in = nc.dram_tensor("ident8_in", [P, P], FP8, kind="ExternalInput")
    idxs_in = nc.dram_tensor("idxs_in", [128, TOTCOLS], I16, kind="ExternalInput")
    s8_in = nc.dram_tensor("s8_in", [128, NPIECES * 128], FP8,
                           kind="ExternalInput")
    out = nc.dram_tensor("out", [cfg.rows_pad, C], F32, kind="ExternalOutput")

    rg = [list(range(cfg.ncores))]

    with tile.TileContext(nc) as tc:
        with tc.tile_pool(name="const", bufs=1) as cp, \
             tc.tile_pool(name="resid", bufs=1) as rp, \
             tc.tile_pool(name="dram", bufs=2, space="DRAM") as dp:

            # ---- constants / residents ----
            identSB = cp.tile([P, P], F32)
            nc.sync.dma_start(identSB[:], ident_in[:])
            ident8SB = cp.tile([P, P], FP8)
            nc.sync.dma_start(ident8SB[:], ident8_in[:])
            w0SB = cp.tile([P, KF, HID], F32)
            nc.sync.dma_start(w0SB[:], w0[:].rearrange("(k p) h -> p k h", p=P))
            w1SB = cp.tile([P, C], F32)
            nc.sync.dma_start(w1SB[:], w1[:])
            b0SB = cp.tile([P, 1], F32)
            nc.sync.dma_start(b0SB[:], b0c[:])
            b1SB = cp.tile([P, C], F32)
            nc.sync.dma_start(b1SB[:], b1r[:])
            dinvSB = cp.tile([P, nwin], F32)
            nc.sync.dma_start(dinvSB[:], dinv_in[:])
            dinvsSB = cp.tile([P, nwin], F32)
            nc.sync.dma_start(dinvsSB[:], dinvs_in[:])
            ddSB = cp.tile([P, nwin], F32)
            nc.sync.dma_start(ddSB[:], dd_in[:])
            zsSB = cp.tile([P, nwin], F32)
            nc.sync.dma_start(zsSB[:], zs_in[:])

            ybSB = rp.tile([P, nwin * C2], BF16)   # y, bf16, 256B padded rows
            zSB = rp.tile([P, nwin * C], BF16)     # alpha*h0/((1-a)*dinv)
            nc.vector.memset(ybSB[:], 0.0)
            # gather indices are identical every iteration: keep resident
            idxAll = rp.tile([128, TOTCOLS], I16)
            nc.sync.dma_start(idxAll[:], idxs_in[:])

            # =========== MLP phase ===========
            ag0 = dp.tile([cfg.rows, C2], BF16)
            with tc.tile_pool(name="mlpw", bufs=3) as wp, \
                 tc.tile_pool(name="mlpp", bufs=2, space="PSUM") as pp:
                for t in range(nwin):
                    xt = wp.tile([P, F_IN], F32, tag="xt")
                    nc.sync.dma_start(xt[:], xin[t * P:(t + 1) * P, :])
                    hT_ps = pp.tile([P, P], F32, tag="hT")
                    for k in range(KF):
                        xT_ps = pp.tile([P, P], F32, tag="xT")
                        nc.tensor.transpose(out=xT_ps[:],
                                            in_=xt[:, k * P:(k + 1) * P],
                                            identity=identSB[:])
                        xT = wp.tile([P, P], F32, tag="xTs")
                        nc.vector.tensor_copy(xT[:], xT_ps[:])
                        nc.tensor.matmul(out=hT_ps[:], lhsT=w0SB[:, k, :],
                                         rhs=xT[:], start=(k == 0),
                                         stop=(k == KF - 1))
                    hT = wp.tile([P, P], F32, tag="hTs")
                    nc.scalar.activation(hT[:], hT_ps[:], AF.Relu,
                                         bias=b0SB[:, 0:1], scale=1.0)
                    h2_ps = pp.tile([P, C], F32, tag="h2")
                    nc.tensor.matmul(out=h2_ps[:], lhsT=hT[:], rhs=w1SB[:],
                                     start=True, stop=True)
                    h0t = wp.tile([P, C], F32, tag="h0t")
                    nc.vector.tensor_tensor(h0t[:], h2_ps[:], b1SB[:], op=ALU.add)
                    nc.scalar.activation(ybSB[:, t * C2:t * C2 + C], h0t[:],
                                         AF.Copy, bias=0.0,
                                         scale=dinvSB[:, t:t + 1])
                    nc.scalar.activation(zSB[:, t * C:(t + 1) * C], h0t[:],
                                         AF.Copy, bias=0.0,
                                         scale=zsSB[:, t:t + 1])
                # write y slice -> ag0
                _dma_slice_to_dram(nc, ag0, ybSB, cfg, 0, nwin, C2)

            yfull = dp.tile([cfg.N, C2], BF16, addr_space="Shared", tag="yfull")
            nc.gpsimd.collective_compute(
                "AllGather", ALU.bypass, replica_groups=rg,
                ins=[ag0[:].opt()], outs=[yfull[:].opt()])

            # =========== APPNP iterations ===========
            for it in range(cfg.K):
                last_it = (it == cfg.K - 1)
                if not last_it:
                    agin = dp.tile([cfg.rows, C2], BF16, tag="agin")
                with tc.tile_pool(name="gpool", bufs=4) as gp, \
                     tc.tile_pool(name="spool", bufs=2) as sp, \
                     tc.tile_pool(name="wk", bufs=6) as wk, \
                     tc.tile_pool(name="pp", bufs=2, space="PSUM") as pp:
                    for b in range(nbatch):
                        wlist = list(range(b * BW, min((b + 1) * BW, nwin)))
                        bcalls = [c for c in calls if c["b"] == b]
                        # stream this batch's S one-hots (fp8) in two halves
                        f0, l0 = batch_sb[b]
                        mid = halfmid[b]
                        sA = sp.tile([P, (mid - f0) * P], FP8, tag="sA")
                        nc.sync.dma_start(sA[:], s8_in[:, f0 * P:mid * P])
                        sB = sp.tile([P, (l0 + 1 - mid) * P], FP8, tag="sB")
                        nc.sync.dma_start(sB[:], s8_in[:, mid * P:(l0 + 1) * P])
                        psumB = pp.tile([P, len(wlist) * C], F32, tag="ps")
                        # teleport + self-loop terms via fp8-identity matmuls.
                        # start=True only on the batch's first matmul into the
                        # bank (clears the whole bank's has_written bits);
                        # per-element has_written makes each region's first
                        # write an overwrite, so no per-window start needed.
                        last_sb = batch_sb[b][1]
                        for w in wlist:
                            wl = w - b * BW
                            osl = psumB[:, wl * C:(wl + 1) * C]
                            nc.tensor.matmul(
                                out=osl, lhsT=ident8SB[:],
                                rhs=zSB[:, w * C:(w + 1) * C],
                                start=(w == wlist[0]), stop=False)
                            nc.tensor.matmul(
                                out=osl, lhsT=ident8SB[:],
                                rhs=ybSB[:, w * C2:w * C2 + C],
                                start=False, stop=False)
                        for call in bcalls:
                            n = call["n"]
                            if n == 0:
                                continue
                            q = call["q"]
                            r = call["r"]
                            col0 = call["col0"]
                            gt = gp.tile([P, (n // P) * C2], BF16, tag="G")
                            src_view = yfull[r * cfg.rng_rows:
                                             min((r + 1) * cfg.rng_rows, cfg.N), :]
                            nc.gpsimd.dma_gather(
                                gt[:].rearrange("p (c f) -> p c f", f=C2),
                                src_view, idxAll[:, col0:col0 + n // 16], n, n,
                                C2, queue_num=q, single_packet=False)
                            for j, ck in enumerate(call["chunks"]):
                                for pc in chunk_pieces[ck]:
                                    sb, wv = pc["sb"], pc["w"]
                                    wl = wv - b * BW
                                    if sb < mid:
                                        ssl = sA[:, (sb - f0) * P:
                                                 (sb - f0 + 1) * P]
                                    else:
                                        ssl = sB[:, (sb - mid) * P:
                                                 (sb - mid + 1) * P]
                                    nc.tensor.matmul(
                                        out=psumB[:, wl * C:(wl + 1) * C],
                                        lhsT=ssl,
                                        rhs=gt[:, j * C2:j * C2 + C],
                                        start=False,
                                        stop=(sb == last_sb))
                        # epilogue per window: single scalar-engine activation
                        for w in wlist:
                            wl = w - b * BW
                            psl = psumB[:, wl * C:(wl + 1) * C]
                            if not last_it:
                                nc.scalar.activation(
                                    ybSB[:, w * C2:w * C2 + C], psl,
                                    AF.Copy, bias=0.0,
                                    scale=ddSB[:, w:w + 1])
                            else:
                                xn = wk.tile([P, C], F32, tag="xn")
                                nc.scalar.activation(
                                    xn[:], psl, AF.Copy, bias=0.0,
                                    scale=dinvsSB[:, w:w + 1])
                                _log_softmax(nc, wk, xn, out, w, cfg)
                        if not last_it:
                            _dma_slice_to_dram(nc, agin, ybSB, cfg, wlist[0],
                                               len(wlist), C2)
                if not last_it:
                    yfull = dp.tile([cfg.N, C2], BF16, addr_space="Shared",
                                    tag="yfull")
                    nc.gpsimd.collective_compute(
                        "AllGather", ALU.bypass, replica_groups=rg,
                        ins=[agin[:].opt()], outs=[yfull[:].opt()])

    nc.compile()
    return nc


def _dma_slice_to_dram(nc, dram_t, ySB, cfg, w0_, nwins, C):
    """Copy windows [w0_, w0_+nwins) of window-major ySB (row width C) into
    row-major dram tensor rows [w0_*P ...], clipping at cfg.rows."""
    end_row = min((w0_ + nwins) * P, cfg.rows)
    n_full = (end_row - w0_ * P) // P
    if n_full > 0:
        dv = dram_t[w0_ * P: w0_ * P + n_full * P, :].rearrange(
            "(w p) c -> p w c", p=P)
        sv = ySB[:, w0_ * C:(w0_ + n_full) * C].rearrange(
            "p (w c) -> p w c", c=C)
        nc.sync.dma_start(dv, sv)
    rem = end_row - (w0_ * P + n_full * P)
    if rem > 0:
        w = w0_ + n_full
        nc.sync.dma_start(dram_t[w * P:w * P + rem, :],
                          ySB[0:rem, w * C:(w + 1) * C])


def _log_softmax(nc, wk, xn, out, w, cfg):
    C = cfg.C
    negm = wk.tile([P, 1], F32, tag="negm")
    nc.vector.reduce_max(negm[:], xn[:], axis=mybir.AxisListType.X,
                         negate=True)
    e = wk.tile([P, C], F32, tag="e")
    ssum = wk.tile([P, 1], F32, tag="ssum")
    nc.scalar.activation(e[:], xn[:], AF.Exp, bias=negm[:, 0:1], scale=1.0,
                         accum_out=ssum[:])
    lse = wk.tile([P, 1], F32, tag="lse")
    nc.scalar.activation(lse[:], ssum[:], AF.Ln)
    res = wk.tile([P, C], F32, tag="res")
    nc.vector.tensor_scalar(res[:], xn[:], negm[:, 0:1], lse[:, 0:1],
                            ALU.add, ALU.subtract)
    nc.sync.dma_start(out[w * P:(w + 1) * P, :], res[:])


# ---------------------------------------------------------------------------
# Host-side driver
# ---------------------------------------------------------------------------

_BUILD_CACHE = {}


def _get_kernel(cfg, edge_index):
    key = hash(edge_index.tobytes()) ^ hash((cfg.N, cfg.E, cfg.K, "v2"))
    if key in _BUILD_CACHE:
        return _BUILD_CACHE[key]
    dinv, idx_t, s8, meta = preprocess(cfg, edge_index)
    nc = build(cfg, meta)
    _BUILD_CACHE[key] = (nc, dinv, idx_t, s8, meta)
    return _BUILD_CACHE[key]


def run(cfg, inputs, edge_index, W0, b0, W1, b1, trace=False):
    nc, dinv, idx_t, s8, meta = _get_kernel(cfg, edge_index)

    ident = np.eye(P, dtype=np.float32)
    import ml_dtypes
    ident8 = np.eye(P, dtype=ml_dtypes.float8_e4m3)
    b0c = np.asarray(b0, np.float32).reshape(P, 1)
    b1r = np.tile(np.asarray(b1, np.float32)[None, :], (P, 1))
    W0 = np.asarray(W0, np.float32)
    W1 = np.asarray(W1, np.float32)
    x = np.asarray(inputs, np.float32)
    a = cfg.ALPHA

    in_maps = []
    for c in range(cfg.ncores):
        r0 = c * cfg.rows
        xs = np.zeros((cfg.rows_pad, cfg.F_IN), np.float32)
        xs[:cfg.rows] = x[r0:r0 + cfg.rows]
        tmp = np.zeros(cfg.rows_pad, np.float32)
        tmp[:cfg.rows] = dinv[r0:r0 + cfg.rows]
        dv = tmp.reshape(cfg.nwin, P).T.copy()   # dv[p, w] = dinv[r0 + w*P + p]
        dvs = dv * (1.0 - a)                     # (1-a)*dinv (final x scale)
        dd = dv * dv * (1.0 - a)                 # (1-a)*dinv^2 (y update)
        with np.errstate(divide="ignore"):
            zs = np.where(dv > 0, a / ((1.0 - a) * dv), 0.0)  # z scale
        in_maps.append(dict(
            xin=xs, w0=W0, w1=W1, b0c=b0c, b1r=b1r,
            dinv_in=dv, dinvs_in=dvs, dd_in=dd, zs_in=zs.astype(np.float32),
            ident_in=ident, ident8_in=ident8,
            idxs_in=idx_t[c], s8_in=s8[c]))

    res = run_bass_kernel_spmd(nc, in_maps, core_ids=list(range(cfg.ncores)),
                               trace=trace)
    outs = [res.results[c]["out"][:cfg.rows] for c in range(cfg.ncores)]
    return np.concatenate(outs, axis=0), res


def kernel(inputs, edge_index, W0, b0, W1, b1):
    cfg = Cfg(N=100000, E=3200000, F_IN=256, HID=128, C=64, K=10, ALPHA=0.1)
    out, _ = run(cfg, np.asarray(inputs), np.asarray(edge_index),
                 W0, b0, W1, b1)
    return out
